# Initial kernel scaffold
#
"""Channel-attention kernel for Trainium2 (8 NeuronCores, batch-parallel).

Reference computation per batch b (feat (C, HW2), word_emb (N, D)):
    we0   = word_emb @ W_fc^T                 (N, HW2)
    S     = feat @ we0^T                      (C, N)   [b_fc shifts every logit
                                                        of a row equally -> the
                                                        softmax is invariant]
    A     = softmax(S, axis=-1)
    out   = A @ we0 + b_fc                    (C, HW2) [b_fc added on host]

Host marshalling: feat is pre-transposed to (HW2, C) per batch and split into
an fp16 hi/lo pair (hi = fp16(x), lo = fp16(x - hi); hi + lo carries ~22
mantissa bits), interleaved per row as [hi(512) | lo(512)] so the DMA reads
2KB-contiguous lines. This puts the contraction dim (hw2) on SBUF partitions
with a plain DMA -- no on-device transposes of the 2 MB feature map -- and
lets the PE run at full fp16 rate (with fast-weight-load) instead of the
4x-slower fp32 path.

Device dataflow per batch (one NeuronCore handles B/8 = 4 batches):
    wn hi/lo    = fp16 split of word_emb    (DVE)
    wembT hi/lo = transposes of wn hi/lo    (PE fp16 transposes)
    we0         = sum of 3 fp16-pair chains wembT^T @ W_fcT  (~fp32-exact)
    we0 hi/lo   = fp16 split of we0; wt hi/lo = transposes   (PE fp16)
    S^T         = wthi^T@FThi + wthi^T@FTlo + wtlo^T@FThi    (~fp32-exact)
    Eh          = exp(0.5*S^T - 48)         (ACT; fixed shift: softmax-exact,
                                             overflow-safe for |logit|<~340)
    E           = Eh*Eh -> float32r         (DVE; = exp(S^T - 96); fp32 range
                                             needed: E spans e^+-80)
    sums        = ones^T @ E                (PE f32r; (1, C) row of softmax
                                             denominators)
    rb          = 1/sums bcast to 77 rows   (DVE reciprocal + GPSIMD
                                             partition_broadcast)
    A^T         = E * rb -> fp16            (DVE; normalized weights in [0,1])
    O           = A-slice^T @ we0h          (PE fp16 + FWL)
    out         = copy O                    (DVE/ACT split, then DMA)

All matmul weight operands are zero-padded to 128 columns so the compiler's
fast-weight-load kicks in; this keeps the PE duty cycle high enough that the
HAM clock-gate stays at full rate.
"""

import numpy as np

import concourse.bass as bass
import concourse.mybir as mybir
import concourse.tile as tile
from concourse import bacc
from concourse.bass import ds, ts
from concourse.bass_utils import run_bass_kernel_spmd
from concourse.masks import make_identity

B, C, HW2 = 32, 512, 1024
N_WORDS, WORD_DIM = 77, 256
H = W = 32
N_CORES = 8
BPC = B // N_CORES  # batches per core

FP32 = mybir.dt.float32
FP16 = mybir.dt.float16
F32R = mybir.dt.float32r
AF = mybir.ActivationFunctionType

EXP_SCALE = 0.5
EXP_BIAS = -48.0  # exp(0.5*s - 48)^2 == exp(s - 96)

LAST_RESULT = None  # BassKernelResults of the most recent run (for test.py)


def _body(nc, tc, ftp_d, wemb_d, wfc_d, out_d):
    from contextlib import ExitStack

    with ExitStack() as ctx:
        const = ctx.enter_context(tc.tile_pool(name="const", bufs=1))
        setup = ctx.enter_context(tc.tile_pool(name="setup", bufs=2))
        big = ctx.enter_context(tc.tile_pool(name="big", bufs=3))
        med = ctx.enter_context(tc.tile_pool(name="med", bufs=3))
        outp = ctx.enter_context(tc.tile_pool(name="outp", bufs=4))
        mm_ps = ctx.enter_context(tc.tile_pool(name="mm_ps", bufs=4, space="PSUM"))
        sm_ps = ctx.enter_context(tc.tile_pool(name="sm_ps", bufs=2, space="PSUM"))
        su_ps = ctx.enter_context(tc.tile_pool(name="su_ps", bufs=1, space="PSUM"))

        ident = const.tile([128, 128], FP32)
        make_identity(nc, ident[:])
        identh = const.tile([128, 128], FP16)
        nc.vector.tensor_copy(identh[:], ident[:])
        ones_f = const.tile([128, 8], FP32)
        nc.gpsimd.memset(ones_f[:], 1.0)
        ones = const.tile([128, 8], F32R)
        nc.vector.tensor_copy(ones[:], ones_f[:])
        ebias = const.tile([128, 1], FP32)
        nc.gpsimd.memset(ebias[:], EXP_BIAS)
        ones1 = const.tile([128, 128], FP32)
        nc.gpsimd.memset(ones1[:], 1.0)

        # ---- W_fc^T (d-partitioned, (2, 128, 1024)), once per core ----
        wfcT = const.tile([128, 2, 1024], FP32)
        wnat0 = setup.tile([128, 8, 256], FP32, tag="wnat0")
        nc.sync.dma_start(wnat0[:], wfc_d.rearrange("(t p) d -> p t d", p=128))
        for kt in range(8):
            for dc in range(2):
                ps = mm_ps.tile([128, 512], FP32, tag="mm")
                nc.tensor.matmul(
                    ps[:, :128],
                    wnat0[:, kt, ts(dc, 128)],
                    ident[:],
                    is_transpose=True,
                )
                nc.vector.tensor_copy(wfcT[:, dc, ts(kt, 128)], ps[:, :128])
        # fp16 hi/lo split of W_fc^T (for the fp16-pair we0 matmul)
        wfcT_hi = const.tile([128, 2, 1024], FP16)
        nc.vector.tensor_copy(wfcT_hi[:], wfcT[:])
        wfcT_lo = const.tile([128, 2, 1024], FP16)
        nc.vector.tensor_sub(wfcT_lo[:], wfcT[:], wfcT_hi[:])

        def load(b):
            # ---- load FT hi|lo (k-partitioned, pre-transposed + interleaved
            #      on host: row k = [hi(512) | lo(512)] -> 2KB DMA lines) ----
            st = {}
            ft = st["ft"] = big.tile([128, 8, 1024], FP16, tag="ft", name="ft")
            nc.sync.dma_start(ft[:], ftp_d[b].rearrange("(t p) x -> p t x", p=128))
            wnat = st["wnat"] = med.tile(
                [128, 256], FP32, tag="wemb_nat", name="wnat"
            )
            nc.sync.dma_start(wnat[:77, :], wemb_d[b])
            return st

        def prep_c(st):
            wnat = st["wnat"]
            # ---- fp16 split of word_emb, then transpose ----
            wnhi = med.tile([128, 256], FP16, tag="wnhi")
            nc.vector.tensor_copy(wnhi[:77, :], wnat[:77, :])
            wnlo = med.tile([128, 256], FP16, tag="wnlo")
            nc.vector.tensor_sub(wnlo[:77, :], wnat[:77, :], wnhi[:77, :])

            # wembT hi/lo (128, 2, 128), zero-padded cols 77:128 for FWL
            wembT_hi = med.tile([128, 2, 128], FP16, tag="wembT_hi")
            wembT_lo = med.tile([128, 2, 128], FP16, tag="wembT_lo")
            nc.gpsimd.memset(wembT_hi[:, :, 77:], 0.0)
            nc.gpsimd.memset(wembT_lo[:, :, 77:], 0.0)
            ps = sm_ps.tile([128, 4, 80], FP16, tag="smallh")
            for j, (src, dc) in enumerate(((wnhi, 0), (wnhi, 1), (wnlo, 0), (wnlo, 1))):
                nc.tensor.matmul(
                    ps[:, j, :77],
                    src[:77, ts(dc, 128)],
                    identh[:77, :77],
                    is_transpose=True,
                    start=(j == 0),
                    stop=(j == 3),
                )
            nc.vector.tensor_copy(wembT_hi[:, :, :77], ps[:, :2, :77])
            nc.vector.tensor_copy(wembT_lo[:, :, :77], ps[:, 2:, :77])

            # ---- we0 = word_emb @ W_fc^T  (77, 1024), fp16-pair chains ----
            we0 = st["we0"] = med.tile([128, 1024], FP32, tag="we0", name="we0")
            for half in range(2):
                ps = mm_ps.tile([128, 512], FP32, tag="mm")
                i_mm = 0
                for dc in range(2):
                    for lhs, rhs in (
                        (wembT_hi, wfcT_hi),
                        (wembT_hi, wfcT_lo),
                        (wembT_lo, wfcT_hi),
                    ):
                        nc.tensor.matmul(
                            ps[:, :],
                            lhs[:, dc, :],
                            rhs[:, dc, ds(half * 512, 512)],
                            start=(i_mm == 0),
                            stop=(i_mm == 5),
                        )
                        i_mm += 1
                nc.scalar.copy(we0[:77, ds(half * 512, 512)], ps[:77, :])
            # fp16 split of we0 for the exact S^T chains; the hi half also
            # serves as the (tolerance-ok) O-matmul rhs
            we0hi = st["we0h"] = med.tile([128, 1024], FP16, tag="we0hi", name="we0hi")
            nc.vector.tensor_copy(we0hi[:77, :], we0[:77, :])
            we0lo = med.tile([128, 1024], FP16, tag="we0lo")
            nc.vector.tensor_sub(we0lo[:77, :], we0[:77, :], we0hi[:77, :])

            # ---- wt hi/lo = we0 hi/lo transposed (8x (128,77) each) ----
            wthi = st["wthi"] = med.tile([128, 8, 128], FP16, tag="wthi", name="wthi")
            wtlo = st["wtlo"] = med.tile([128, 8, 128], FP16, tag="wtlo", name="wtlo")
            nc.gpsimd.memset(wthi[:, :, 77:], 0.0)
            nc.gpsimd.memset(wtlo[:, :, 77:], 0.0)
            for src, dst in ((we0hi, wthi), (we0lo, wtlo)):
                for g in range(2):
                    ps = sm_ps.tile([128, 4, 80], FP16, tag="smallh")
                    for j in range(4):
                        nc.tensor.matmul(
                            ps[:, j, :77],
                            src[:77, ts(g * 4 + j, 128)],
                            identh[:77, :77],
                            is_transpose=True,
                            start=(j == 0),
                            stop=(j == 3),
                        )
                    nc.vector.tensor_copy(dst[:, ds(g * 4, 4), :77], ps[:, :, :77])
            return st

        def score(st):
            # ---- S^T = wt^T @ FT  (77, 512), 3 fp16 chains ----
            ft, wthi, wtlo = st["ft"], st["wthi"], st["wtlo"]
            sps = st["sps"] = mm_ps.tile([128, 512], FP32, tag="mm", name="sps")
            n_mm = 24
            i_mm = 0
            for kt in range(8):
                for lhs, sl in (
                    (wthi, ds(0, 512)),  # hi @ hi
                    (wthi, ds(512, 512)),  # hi @ lo (same weights)
                    (wtlo, ds(0, 512)),  # lo @ hi
                ):
                    nc.tensor.matmul(
                        sps[:, :],
                        lhs[:, kt, :],
                        ft[:, kt, sl],
                        start=(i_mm == 0),
                        stop=(i_mm == n_mm - 1),
                    )
                    i_mm += 1

        def soft(st):
            # ---- E = exp(S^T - 96), via exp(0.5 s - 48)^2 ----
            sps = st["sps"]
            ehalf = med.tile([128, 512], FP32, tag="ehalf")
            nc.scalar.activation(
                ehalf[:77, :], sps[:77, :], AF.Exp, bias=ebias[:77, :], scale=EXP_SCALE
            )
            eT = st["eT"] = med.tile([128, 512], F32R, tag="eT", name="eT")
            nc.vector.tensor_mul(eT[:77, :], ehalf[:77, :], ehalf[:77, :])

        def sums_a(st):
            # ---- softmax denominators: (1, C) row, then 1/row ----
            eT = st["eT"]
            sus = su_ps.tile([128, 512], FP32, tag="sums")
            nc.tensor.matmul(sus[:8, :], ones[:77, :], eT[:77, :])
            # 1/sums on the single-partition row (approx: ~18 bits, far below
            # the fp16 rounding of A)
            rrow = st["rrow"] = med.tile([128, 512], FP32, tag="rrow", name="rrow")
            nc.vector.reciprocal_approx_fast(rrow[:1, :], sus[:1, :])

        def sums_b(st):
            # ---- fan 1/sums out to 77 rows (K=1 PE matmul), A = E/sums ----
            eT, rrow = st["eT"], st["rrow"]
            rb = su_ps.tile([128, 512], FP32, tag="rb")
            nc.tensor.matmul(rb[:77, :], ones1[:1, :77], rrow[:1, :])
            at = st["at"] = med.tile([128, 512], FP16, tag="at", name="at")
            nc.vector.tensor_mul(at[:77, :], eT[:77, :], rb[:77, :])

        def o_phase(st, b):
            # ---- per c-tile: O = A-slice^T @ we0hi, copy out, store ----
            at, we0h = st["at"], st["we0h"]
            for ct in range(4):
                ops0 = mm_ps.tile([128, 512], FP32, tag="mm")
                nc.tensor.matmul(ops0[:], at[:77, ts(ct, 128)], we0h[:77, :512])
                ops1 = mm_ps.tile([128, 512], FP32, tag="mm")
                nc.tensor.matmul(ops1[:], at[:77, ts(ct, 128)], we0h[:77, 512:])
                ob = outp.tile([128, 1024], FP32, tag="outb")
                # split the PSUM->SBUF moves between DVE and ACT
                nc.vector.tensor_copy(ob[:, :512], ops0[:])
                nc.scalar.copy(ob[:, 512:], ops1[:])
                nc.sync.dma_start(out_d[b, ts(ct, 128), :], ob[:])

        # software pipeline: batch b's normalize + output phases are emitted
        # behind batch b+1's prep/score, so the (in-order) PE queue always has
        # independent work while b's softmax chain runs on ACT/GPSIMD/DVE --
        # keeps the PE HAM-warm.
        states = {}
        states[0] = load(0)
        prep_c(states[0])
        states[1] = load(1)
        score(states[0])
        soft(states[0])
        for b in range(1, BPC):
            sums_a(states[b - 1])
            prep_c(states[b])
            if b + 1 < BPC:
                states[b + 1] = load(b + 1)
            sums_b(states[b - 1])
            score(states[b])
            o_phase(states[b - 1], b - 1)
            del states[b - 1]
            soft(states[b])
        sums_a(states[BPC - 1])
        sums_b(states[BPC - 1])
        o_phase(states[BPC - 1], BPC - 1)


def _build():
    nc = bacc.Bacc(
        "TRN2",
        target_bir_lowering=False,
        debug=False,
        enable_asserts=False,
        num_devices=N_CORES,
    )
    ftp_d = nc.declare_dram_parameter("ftp", [BPC, HW2, 2 * C], FP16, isOutput=False)
    wemb_d = nc.declare_dram_parameter(
        "wemb", [BPC, N_WORDS, WORD_DIM], FP32, isOutput=False
    )
    wfc_d = nc.declare_dram_parameter("wfc", [HW2, WORD_DIM], FP32, isOutput=False)
    out_d = nc.declare_dram_parameter("out", [BPC, C, HW2], FP32, isOutput=True)
    with tile.TileContext(nc) as tc:
        _body(nc, tc, ftp_d, wemb_d, wfc_d, out_d)
    nc.finalize()
    return nc


_CACHE = {}


def kernel(feat, word_emb, W_fc, b_fc, **run_kwargs):
    global LAST_RESULT
    feat = np.asarray(feat, dtype=np.float32).reshape(B, C, HW2)
    word_emb = np.ascontiguousarray(np.asarray(word_emb, dtype=np.float32))
    W_fc = np.ascontiguousarray(np.asarray(W_fc, dtype=np.float32))
    b_fc = np.asarray(b_fc, dtype=np.float32)

    # host marshalling: transpose to (B, HW2, C); split into fp16 hi+lo,
    # interleaved per row as [hi(512) | lo(512)] for 2KB-contiguous DMA lines
    featT = np.ascontiguousarray(feat.transpose(0, 2, 1))
    fthi = featT.astype(np.float16)
    ftlo = (featT - fthi.astype(np.float32)).astype(np.float16)
    ftp = np.empty((B, HW2, 2 * C), dtype=np.float16)
    ftp[:, :, :C] = fthi
    ftp[:, :, C:] = ftlo

    if "nc" not in _CACHE:
        _CACHE["nc"] = _build()
    nc = _CACHE["nc"]

    in_maps = [
        {
            "ftp": ftp[i * BPC : (i + 1) * BPC],
            "wemb": word_emb[i * BPC : (i + 1) * BPC],
            "wfc": W_fc,
        }
        for i in range(N_CORES)
    ]
    res = run_bass_kernel_spmd(nc, in_maps, list(range(N_CORES)), **run_kwargs)
    LAST_RESULT = res
    out = np.concatenate([res.results[i]["out"] for i in range(N_CORES)], axis=0)
    # b_fc shifts all logits of a softmax row equally (no effect on A) and
    # adds linearly to the output: out = A @ we0 + b_fc. Exact identity.
    out = out + b_fc.reshape(1, 1, HW2)
    return out.reshape(B, C, H, W).astype(np.float32)



# revision 7
# speedup vs baseline: 1.4719x; 1.4719x over previous
"""Channel-attention kernel for Trainium2 (8 NeuronCores, batch-parallel).

Reference computation per batch b (feat (C, HW2), word_emb (N, D)):
    we0   = word_emb @ W_fc^T                 (N, HW2)
    S     = feat @ we0^T                      (C, N)   [b_fc shifts every logit
                                                        of a row equally -> the
                                                        softmax is invariant]
    A     = softmax(S, axis=-1)
    out   = A @ we0 + b_fc                    (C, HW2) [b_fc added on host]

v3 design (v1 108.6us -> v2 65.5us -> this):
  - feat ships fp16-hi ONLY (half the input DMA); score = 2 fp16 chains.
  - output stored fp16 (half the output DMA); host casts to fp32.
  - softmax normalization folded into the O-phase: O = E^T @ we0 as f32r
    matmuls with UN-normalized E stationary; per-c sums via a tiny ones
    matmul; 1/sums applied as a per-partition scale during the PSUM->SBUF
    out-copies (ACT + DVE split).
  - we0 transposed once in fp32 (8 PE transposes), hi/lo split from PSUM.
  - wembT/wfcT hi/lo marshalled on host.
  - v3: O(b-1) PE work emitted between we0(b) and wt(b) so the PE stays busy
    while ACT drains we0(b) copies; one out-DMA per batch instead of 4; we0
    PSUM merged to a single 2-bank tile with one ACT copy; vt loaded before
    ft0 so batch 0 starts ~3us earlier.
  Numerics (numpy-emulated, real seed): scale-rel absmax ~7.8e-3 (gate 2e-2);
  v2 measured 7.52e-3.

Device dataflow per batch (one NeuronCore handles B/8 = 4 batches):
    we0 psum    = 3 fp16-pair chains wembT^T @ wfcT      (12 mm, ~fp32-exact)
    we0         = ACT copy psum -> SBUF f32r             (O-phase moving)
    we0T psum   = 8 fp32 PE transposes of we0 (bitcast)
    wt hi       = ACT copy we0T psum -> fp16
    wt lo       = DVE sub (we0T psum - wt hi) -> fp16
    S^T         = wthi^T@FThi + wtlo^T@FThi              (16 mm into one PSUM)
    Eh          = exp(0.5*S^T - 48)                      (ACT)
    E           = Eh*Eh -> f32r                          (DVE; = exp(S^T-96))
    per ct(4):  O = E-slice^T @ we0 (f32r, 2 mm), sums = E-slice^T @ ones
                rr = 1/sums (DVE); out fp16 = psum * rr (ACT half, DVE half)
    out         = one DMA (128, 4, 1024) via the scalar queue
"""

import numpy as np

import concourse.bass as bass
import concourse.mybir as mybir
import concourse.tile as tile
from concourse import bacc
from concourse.bass import ds, ts
from concourse.bass_utils import run_bass_kernel_spmd
from concourse.masks import make_identity

B, C, HW2 = 32, 512, 1024
N_WORDS, WORD_DIM = 77, 256
H = W = 32
N_CORES = 8
BPC = B // N_CORES  # batches per core
NP = 80  # N_WORDS padded to a multiple of 16

FP32 = mybir.dt.float32
FP16 = mybir.dt.float16
F32R = mybir.dt.float32r
AF = mybir.ActivationFunctionType

EXP_SCALE = 0.5
EXP_BIAS = -48.0  # exp(0.5*s - 48)^2 == exp(s - 96)

LAST_RESULT = None  # BassKernelResults of the most recent run (for test.py)


def _body(nc, tc, ftp_d, wemb_d, wfc_d, out_d):
    from contextlib import ExitStack

    with ExitStack() as ctx:
        const = ctx.enter_context(tc.tile_pool(name="const", bufs=1))
        big = ctx.enter_context(tc.tile_pool(name="big", bufs=2))
        med = ctx.enter_context(tc.tile_pool(name="med", bufs=2))
        outp = ctx.enter_context(tc.tile_pool(name="outp", bufs=2))
        mm_ps = ctx.enter_context(tc.tile_pool(name="mm_ps", bufs=2, space="PSUM"))
        tp_ps = ctx.enter_context(tc.tile_pool(name="tp_ps", bufs=2, space="PSUM"))
        sc_ps = ctx.enter_context(tc.tile_pool(name="sc_ps", bufs=1, space="PSUM"))
        su_ps = ctx.enter_context(tc.tile_pool(name="su_ps", bufs=1, space="PSUM"))

        ident = const.tile([128, 128], FP32)
        make_identity(nc, ident[:])
        ones_f = const.tile([128, 8], FP32)
        nc.gpsimd.memset(ones_f[:], 1.0)
        ones = const.tile([128, 8], F32R)
        nc.vector.tensor_copy(ones[:], ones_f[:])
        ebias = const.tile([128, 1], FP32)
        nc.gpsimd.memset(ebias[:], EXP_BIAS)

        # W_fc^T hi|lo packed, host-marshalled: (256, 2048) -> (128, 2, 2048)
        vt = const.tile([128, 2, 2048], FP16)

        def load_wemb(b, st):
            # wembT packed (256, 160) = [hi(80) | lo(80)] -> (128, 2, 160)
            wembT = st["wembT"] = med.tile(
                [128, 2, 160], FP16, tag="wembT", name="wembT"
            )
            nc.sync.dma_start(wembT[:], wemb_d[b].rearrange("(c p) x -> p c x", p=128))

        def load_ft(b, st):
            # FT hi (hw-partitioned): (1024, 512) -> (128, 8, 512)
            ft = st["ft"] = big.tile([128, 8, 512], FP16, tag="ft", name="ft")
            nc.sync.dma_start(ft[:], ftp_d[b].rearrange("(t p) x -> p t x", p=128))

        def we0_phase(st):
            # we0 = word_emb @ W_fc^T (77, 1024): 3 fp16-pair chains into one
            # 2-bank PSUM tile; vhi-only chains first so batch 0 can start
            # before the vlo half of the wfc DMA lands.
            wembT = st["wembT"]
            whi = (wembT[:, 0, :NP], wembT[:, 1, :NP])
            wlo = (wembT[:, 0, NP:], wembT[:, 1, NP:])
            ps = st["we0ps"] = mm_ps.tile([128, 1024], FP32, tag="mm", name="we0ps")
            mms = []  # (lhsT, dc, vt_base, half)
            for half in range(2):
                for dc in range(2):
                    mms.append((whi[dc], dc, 0, half))
            for half in range(2):
                for dc in range(2):
                    mms.append((whi[dc], dc, 1024, half))
                    mms.append((wlo[dc], dc, 0, half))
            seen = [0, 0]
            for lhsT, dc, base, half in mms:
                seen[half] += 1
                nc.tensor.matmul(
                    ps[:NP, ds(half * 512, 512)],
                    lhsT,
                    vt[:, dc, ds(base + half * 512, 512)],
                    start=(seen[half] == 1),
                    stop=(seen[half] == 6),
                    skip_group_check=True,
                )
            we0 = st["we0"] = med.tile([128, 1024], F32R, tag="we0", name="we0")
            nc.scalar.copy(we0[:N_WORDS, :], ps[:N_WORDS, :])

        def wt_phase(st):
            # we0^T via 8 fp32 PE transposes; split hi/lo straight from PSUM
            we0 = st["we0"][:].bitcast(FP32)
            wthi = st["wthi"] = med.tile([128, 8, NP], FP16, tag="wthi", name="wthi")
            wtlo = st["wtlo"] = med.tile([128, 8, NP], FP16, tag="wtlo", name="wtlo")
            for g in range(2):
                tps = tp_ps.tile([128, 4, NP], FP32, tag="tp")
                for j in range(4):
                    nc.tensor.matmul(
                        tps[:, j, :N_WORDS],
                        we0[:N_WORDS, ts(g * 4 + j, 128)],
                        ident[:N_WORDS, :N_WORDS],
                        is_transpose=True,
                        start=(j == 0),
                        stop=(j == 3),
                    )
                gs = ds(g * 4, 4)
                nc.scalar.copy(wthi[:, gs, :N_WORDS], tps[:, :, :N_WORDS])
                nc.vector.tensor_sub(
                    wtlo[:, gs, :N_WORDS], tps[:, :, :N_WORDS], wthi[:, gs, :N_WORDS]
                )

        def score(st):
            # S^T = wthi^T @ FThi + wtlo^T @ FThi  (77, 512), 16 matmuls
            ft = st["ft"]
            sps = st["sps"] = sc_ps.tile([128, 512], FP32, tag="sc", name="sps")
            i = 0
            for src in (st["wthi"], st["wtlo"]):
                for kt in range(8):
                    nc.tensor.matmul(
                        sps[:NP, :],
                        src[:, kt, :],
                        ft[:, kt, :],
                        start=(i == 0),
                        stop=(i == 15),
                    )
                    i += 1

        def soft(st):
            # E = exp(S^T - 96) via exp(0.5*s - 48)^2, kept f32r un-normalized
            sps = st["sps"]
            ehalf = med.tile([128, 512], FP32, tag="ehalf")
            nc.scalar.activation(
                ehalf[:N_WORDS, :],
                sps[:N_WORDS, :],
                AF.Exp,
                bias=ebias[:N_WORDS, :],
                scale=EXP_SCALE,
            )
            eT = st["eT"] = med.tile([128, 512], F32R, tag="eT", name="eT")
            nc.vector.tensor_mul(eT[:N_WORDS, :], ehalf[:N_WORDS, :], ehalf[:N_WORDS, :])

        def o_phase(st, b):
            # O = E-slice^T @ we0 (f32r) + ones-column sums; normalize on copy
            eT, we0 = st["eT"], st["we0"]
            rr = med.tile([128, 4], FP32, tag="rr")
            ob = outp.tile([128, 4, 1024], FP16, tag="ob")
            for ct in range(4):
                e_sl = eT[:N_WORDS, ts(ct, 128)]
                ops = mm_ps.tile([128, 1024], FP32, tag="mm")
                nc.tensor.matmul(ops[:, :512], e_sl, we0[:N_WORDS, :512])
                nc.tensor.matmul(ops[:, 512:], e_sl, we0[:N_WORDS, 512:])
                su = su_ps.tile([128, 8], FP32, tag="su")
                nc.tensor.matmul(su[:], e_sl, ones[:N_WORDS, :])
                rc = rr[:, ct : ct + 1]
                nc.vector.reciprocal(rc, su[:, 0:1])
                nc.scalar.mul(ob[:, ct, :512], ops[:, :512], rc)
                nc.vector.tensor_scalar_mul(ob[:, ct, 512:], ops[:, 512:], rc)
            nc.scalar.dma_start(
                out_d[b].rearrange("(ct p) x -> p ct x", p=128), ob[:]
            )

        # software pipeline: O(b-1) PE work is emitted right after we0(b)'s
        # matmuls so the PE streams O while ACT drains we0(b)'s PSUM copies
        # (which gate the transposes and score of batch b).
        states = {b: {} for b in range(BPC)}
        load_wemb(0, states[0])
        nc.sync.dma_start(
            vt[:, :, :1024], wfc_d[:, :1024].rearrange("(c p) x -> p c x", p=128)
        )
        nc.sync.dma_start(
            vt[:, :, 1024:], wfc_d[:, 1024:].rearrange("(c p) x -> p c x", p=128)
        )
        load_ft(0, states[0])
        we0_phase(states[0])
        wt_phase(states[0])
        load_wemb(1, states[1])
        load_ft(1, states[1])
        score(states[0])
        soft(states[0])
        for b in range(1, BPC):
            we0_phase(states[b])
            o_phase(states[b - 1], b - 1)
            wt_phase(states[b])
            if b + 1 < BPC:
                load_wemb(b + 1, states[b + 1])
                load_ft(b + 1, states[b + 1])
            score(states[b])
            soft(states[b])
            del states[b - 1]
        o_phase(states[BPC - 1], BPC - 1)


def _build():
    nc = bacc.Bacc(
        "TRN2",
        target_bir_lowering=False,
        debug=False,
        enable_asserts=False,
        num_devices=N_CORES,
    )
    ftp_d = nc.declare_dram_parameter("ftp", [BPC, HW2, C], FP16, isOutput=False)
    wemb_d = nc.declare_dram_parameter(
        "wemb", [BPC, WORD_DIM, 2 * NP], FP16, isOutput=False
    )
    wfc_d = nc.declare_dram_parameter("wfc", [WORD_DIM, 2048], FP16, isOutput=False)
    out_d = nc.declare_dram_parameter("out", [BPC, C, HW2], FP16, isOutput=True)
    with tile.TileContext(nc) as tc:
        _body(nc, tc, ftp_d, wemb_d, wfc_d, out_d)
    nc.finalize()
    return nc


_CACHE = {}


def kernel(feat, word_emb, W_fc, b_fc, **run_kwargs):
    global LAST_RESULT
    feat = np.asarray(feat, dtype=np.float32).reshape(B, C, HW2)
    word_emb = np.asarray(word_emb, dtype=np.float32)
    W_fc = np.asarray(W_fc, dtype=np.float32)
    b_fc = np.asarray(b_fc, dtype=np.float32)

    # host marshalling (layout/dtype only):
    # feat -> fp16 hi of feat^T, (B, HW2, C)
    ftp = np.ascontiguousarray(feat.transpose(0, 2, 1)).astype(np.float16)
    # word_emb^T hi/lo packed (B, 256, 160) = [hi(77->80) | lo(77->80)]
    wembT = np.ascontiguousarray(word_emb.transpose(0, 2, 1))  # (B, 256, 77)
    whi = wembT.astype(np.float16)
    wlo = (wembT - whi.astype(np.float32)).astype(np.float16)
    wembp = np.zeros((B, WORD_DIM, 2 * NP), dtype=np.float16)
    wembp[:, :, :N_WORDS] = whi
    wembp[:, :, NP : NP + N_WORDS] = wlo
    # W_fc^T hi/lo packed (256, 2048) = [hi(1024) | lo(1024)]
    vT = np.ascontiguousarray(W_fc.T)  # (256, 1024)
    vhi = vT.astype(np.float16)
    vlo = (vT - vhi.astype(np.float32)).astype(np.float16)
    wfcp = np.concatenate([vhi, vlo], axis=1)  # (256, 2048)

    if "nc" not in _CACHE:
        _CACHE["nc"] = _build()
    nc = _CACHE["nc"]

    in_maps = [
        {
            "ftp": ftp[i * BPC : (i + 1) * BPC],
            "wemb": wembp[i * BPC : (i + 1) * BPC],
            "wfc": wfcp,
        }
        for i in range(N_CORES)
    ]
    res = run_bass_kernel_spmd(nc, in_maps, list(range(N_CORES)), **run_kwargs)
    LAST_RESULT = res
    out = np.concatenate([res.results[i]["out"] for i in range(N_CORES)], axis=0)
    # b_fc shifts all logits of a softmax row equally (no effect on A) and
    # adds linearly to the output: out = A @ we0 + b_fc. Exact identity.
    out = out.astype(np.float32) + b_fc.reshape(1, 1, HW2)
    return out.reshape(B, C, H, W).astype(np.float32)


# revision 8
# speedup vs baseline: 1.5207x; 1.0332x over previous
"""Channel-attention kernel for Trainium2 (8 NeuronCores, batch-parallel).

Reference computation per batch b (feat (C, HW2), word_emb (N, D)):
    we0   = word_emb @ W_fc^T                 (N, HW2)
    S     = feat @ we0^T                      (C, N)   [b_fc shifts every logit
                                                        of a row equally -> the
                                                        softmax is invariant]
    A     = softmax(S, axis=-1)
    out   = A @ we0 + b_fc                    (C, HW2) [b_fc added on host]

v3 design (v1 108.6us -> v2 65.5us -> this):
  - feat ships fp16-hi ONLY (half the input DMA); score = 2 fp16 chains.
  - output stored fp16 (half the output DMA); host casts to fp32.
  - softmax normalization folded into the O-phase: O = E^T @ we0 as f32r
    matmuls with UN-normalized E stationary; per-c sums via a tiny ones
    matmul; 1/sums applied as a per-partition scale during the PSUM->SBUF
    out-copies (ACT + DVE split).
  - we0 transposed once in fp32 (8 PE transposes), hi/lo split from PSUM.
  - wembT/wfcT hi/lo marshalled on host.
  - v3: O(b-1) PE work emitted between we0(b) and wt(b) so the PE stays busy
    while ACT drains we0(b) copies; one out-DMA per batch instead of 4; we0
    PSUM merged to a single 2-bank tile with one ACT copy; vt loaded before
    ft0 so batch 0 starts ~3us earlier.
  Numerics (numpy-emulated, real seed): scale-rel absmax ~7.8e-3 (gate 2e-2);
  v2 measured 7.52e-3.

Device dataflow per batch (one NeuronCore handles B/8 = 4 batches):
    we0 psum    = 3 fp16-pair chains wembT^T @ wfcT      (12 mm, ~fp32-exact)
    we0         = ACT copy psum -> SBUF f32r             (O-phase moving)
    we0T psum   = 8 fp32 PE transposes of we0 (bitcast)
    wt hi       = ACT copy we0T psum -> fp16
    wt lo       = DVE sub (we0T psum - wt hi) -> fp16
    S^T         = wthi^T@FThi + wtlo^T@FThi              (16 mm into one PSUM)
    Eh          = exp(0.5*S^T - 48)                      (ACT)
    E           = Eh*Eh -> f32r                          (DVE; = exp(S^T-96))
    per ct(4):  O = E-slice^T @ we0 (f32r, 2 mm), sums = E-slice^T @ ones
                rr = 1/sums (DVE); out fp16 = psum * rr (ACT half, DVE half)
    out         = one DMA (128, 4, 1024) via the scalar queue
"""

import numpy as np

import concourse.bass as bass
import concourse.mybir as mybir
import concourse.tile as tile
from concourse import bacc
from concourse.bass import ds, ts
from concourse.bass_utils import run_bass_kernel_spmd
from concourse.masks import make_identity

B, C, HW2 = 32, 512, 1024
N_WORDS, WORD_DIM = 77, 256
H = W = 32
N_CORES = 8
BPC = B // N_CORES  # batches per core
NP = 80  # N_WORDS padded to a multiple of 16

FP32 = mybir.dt.float32
FP16 = mybir.dt.float16
F32R = mybir.dt.float32r
AF = mybir.ActivationFunctionType

EXP_SCALE = 0.5
EXP_BIAS = -48.0  # exp(0.5*s - 48)^2 == exp(s - 96)

LAST_RESULT = None  # BassKernelResults of the most recent run (for test.py)


def _body(nc, tc, ftp_d, wemb_d, wfc_d, out_d):
    from contextlib import ExitStack

    with ExitStack() as ctx:
        const = ctx.enter_context(tc.tile_pool(name="const", bufs=1))
        big = ctx.enter_context(tc.tile_pool(name="big", bufs=2))
        med = ctx.enter_context(tc.tile_pool(name="med", bufs=2))
        outp = ctx.enter_context(tc.tile_pool(name="outp", bufs=2))
        mm_ps = ctx.enter_context(tc.tile_pool(name="mm_ps", bufs=2, space="PSUM"))
        tp_ps = ctx.enter_context(tc.tile_pool(name="tp_ps", bufs=2, space="PSUM"))
        sc_ps = ctx.enter_context(tc.tile_pool(name="sc_ps", bufs=1, space="PSUM"))
        su_ps = ctx.enter_context(tc.tile_pool(name="su_ps", bufs=1, space="PSUM"))

        ident = const.tile([128, 128], FP32)
        make_identity(nc, ident[:])
        ones_f = const.tile([128, 8], FP32)
        nc.gpsimd.memset(ones_f[:], 1.0)
        ones = const.tile([128, 8], F32R)
        nc.vector.tensor_copy(ones[:], ones_f[:])
        ebias = const.tile([128, 1], FP32)
        nc.gpsimd.memset(ebias[:], EXP_BIAS)

        # W_fc^T hi|lo packed, host-marshalled: (256, 2048) -> (128, 2, 2048)
        vt = const.tile([128, 2, 2048], FP16)

        def load_wemb(b, st):
            # wembT packed (256, 160) = [hi(80) | lo(80)] -> (128, 2, 160)
            wembT = st["wembT"] = med.tile(
                [128, 2, 160], FP16, tag="wembT", name="wembT"
            )
            nc.sync.dma_start(wembT[:], wemb_d[b].rearrange("(c p) x -> p c x", p=128))

        def load_ft(b, st):
            # FT hi (hw-partitioned): (1024, 512) -> (128, 8, 512)
            ft = st["ft"] = big.tile([128, 8, 512], FP16, tag="ft", name="ft")
            nc.sync.dma_start(ft[:], ftp_d[b].rearrange("(t p) x -> p t x", p=128))

        def we0_phase(st):
            # we0 = word_emb @ W_fc^T (77, 1024): 3 fp16-pair chains into one
            # 2-bank PSUM tile; vhi-only chains first so batch 0 can start
            # before the vlo half of the wfc DMA lands.
            wembT = st["wembT"]
            whi = (wembT[:, 0, :NP], wembT[:, 1, :NP])
            wlo = (wembT[:, 0, NP:], wembT[:, 1, NP:])
            ps = st["we0ps"] = mm_ps.tile([128, 1024], FP32, tag="mm", name="we0ps")
            mms = []  # (lhsT, dc, vt_base, half)
            for half in range(2):
                for dc in range(2):
                    mms.append((whi[dc], dc, 0, half))
            for half in range(2):
                for dc in range(2):
                    mms.append((whi[dc], dc, 1024, half))
                    mms.append((wlo[dc], dc, 0, half))
            seen = [0, 0]
            for lhsT, dc, base, half in mms:
                seen[half] += 1
                nc.tensor.matmul(
                    ps[:NP, ds(half * 512, 512)],
                    lhsT,
                    vt[:, dc, ds(base + half * 512, 512)],
                    start=(seen[half] == 1),
                    stop=(seen[half] == 6),
                    skip_group_check=True,
                )
            we0 = st["we0"] = med.tile([128, 1024], F32R, tag="we0", name="we0")
            nc.scalar.copy(we0[:N_WORDS, :], ps[:N_WORDS, :])

        def wt_phase(st):
            # we0^T via 8 fp32 PE transposes; split hi/lo straight from PSUM
            we0 = st["we0"][:].bitcast(FP32)
            wthi = st["wthi"] = med.tile([128, 8, NP], FP16, tag="wthi", name="wthi")
            wtlo = st["wtlo"] = med.tile([128, 8, NP], FP16, tag="wtlo", name="wtlo")
            for g in range(2):
                tps = tp_ps.tile([128, 4, NP], FP32, tag="tp")
                for j in range(4):
                    nc.tensor.matmul(
                        tps[:, j, :N_WORDS],
                        we0[:N_WORDS, ts(g * 4 + j, 128)],
                        ident[:N_WORDS, :N_WORDS],
                        is_transpose=True,
                        start=(j == 0),
                        stop=(j == 3),
                    )
                gs = ds(g * 4, 4)
                nc.scalar.copy(wthi[:, gs, :N_WORDS], tps[:, :, :N_WORDS])
                nc.vector.tensor_sub(
                    wtlo[:, gs, :N_WORDS], tps[:, :, :N_WORDS], wthi[:, gs, :N_WORDS]
                )

        def score(st):
            # S^T = wthi^T @ FThi + wtlo^T @ FThi  (77, 512), 16 matmuls
            ft = st["ft"]
            sps = st["sps"] = sc_ps.tile([128, 512], FP32, tag="sc", name="sps")
            i = 0
            for src in (st["wthi"], st["wtlo"]):
                for kt in range(8):
                    nc.tensor.matmul(
                        sps[:NP, :],
                        src[:, kt, :],
                        ft[:, kt, :],
                        start=(i == 0),
                        stop=(i == 15),
                    )
                    i += 1

        def soft(st):
            # E = exp(S^T - 96) via exp(0.5*s - 48)^2, kept f32r un-normalized
            sps = st["sps"]
            ehalf = med.tile([128, 512], FP32, tag="ehalf")
            nc.scalar.activation(
                ehalf[:N_WORDS, :],
                sps[:N_WORDS, :],
                AF.Exp,
                bias=ebias[:N_WORDS, :],
                scale=EXP_SCALE,
            )
            eT = st["eT"] = med.tile([128, 512], F32R, tag="eT", name="eT")
            nc.vector.tensor_mul(eT[:N_WORDS, :], ehalf[:N_WORDS, :], ehalf[:N_WORDS, :])

        def o_phase(st, b):
            # O = E-slice^T @ we0 (f32r) + ones-column sums; normalize on copy
            eT, we0 = st["eT"], st["we0"]
            rr = med.tile([128, 4], FP32, tag="rr")
            ob = outp.tile([128, 4, 1024], FP16, tag="ob")
            for ct in range(4):
                e_sl = eT[:N_WORDS, ts(ct, 128)]
                ops = mm_ps.tile([128, 1024], FP32, tag="mm")
                nc.tensor.matmul(ops[:, :512], e_sl, we0[:N_WORDS, :512])
                nc.tensor.matmul(ops[:, 512:], e_sl, we0[:N_WORDS, 512:])
                su = su_ps.tile([128, 8], FP32, tag="su")
                nc.tensor.matmul(su[:], e_sl, ones[:N_WORDS, :])
                rc = rr[:, ct : ct + 1]
                nc.vector.reciprocal(rc, su[:, 0:1])
                nc.scalar.mul(ob[:, ct, :512], ops[:, :512], rc)
                nc.vector.tensor_scalar_mul(ob[:, ct, 512:], ops[:, 512:], rc)
            nc.scalar.dma_start(
                out_d[b].rearrange("(ct p) x -> p ct x", p=128), ob[:]
            )

        # software pipeline: O(b-1) PE work is emitted right after we0(b)'s
        # matmuls so the PE streams O while ACT drains we0(b)'s PSUM copies
        # (which gate the transposes and score of batch b).
        states = {b: {} for b in range(BPC)}
        load_wemb(0, states[0])
        nc.sync.dma_start(
            vt[:, :, :1024], wfc_d[:, :1024].rearrange("(c p) x -> p c x", p=128)
        )
        nc.sync.dma_start(
            vt[:, :, 1024:], wfc_d[:, 1024:].rearrange("(c p) x -> p c x", p=128)
        )
        load_ft(0, states[0])
        we0_phase(states[0])
        wt_phase(states[0])
        load_wemb(1, states[1])
        load_ft(1, states[1])
        score(states[0])
        soft(states[0])
        for b in range(1, BPC):
            we0_phase(states[b])
            wt_phase(states[b])
            o_phase(states[b - 1], b - 1)
            if b + 1 < BPC:
                load_wemb(b + 1, states[b + 1])
                load_ft(b + 1, states[b + 1])
            score(states[b])
            soft(states[b])
            del states[b - 1]
        o_phase(states[BPC - 1], BPC - 1)


def _build():
    nc = bacc.Bacc(
        "TRN2",
        target_bir_lowering=False,
        debug=False,
        enable_asserts=False,
        num_devices=N_CORES,
    )
    ftp_d = nc.declare_dram_parameter("ftp", [BPC, HW2, C], FP16, isOutput=False)
    wemb_d = nc.declare_dram_parameter(
        "wemb", [BPC, WORD_DIM, 2 * NP], FP16, isOutput=False
    )
    wfc_d = nc.declare_dram_parameter("wfc", [WORD_DIM, 2048], FP16, isOutput=False)
    out_d = nc.declare_dram_parameter("out", [BPC, C, HW2], FP16, isOutput=True)
    with tile.TileContext(nc) as tc:
        _body(nc, tc, ftp_d, wemb_d, wfc_d, out_d)
    nc.finalize()
    return nc


_CACHE = {}


def kernel(feat, word_emb, W_fc, b_fc, **run_kwargs):
    global LAST_RESULT
    feat = np.asarray(feat, dtype=np.float32).reshape(B, C, HW2)
    word_emb = np.asarray(word_emb, dtype=np.float32)
    W_fc = np.asarray(W_fc, dtype=np.float32)
    b_fc = np.asarray(b_fc, dtype=np.float32)

    # host marshalling (layout/dtype only):
    # feat -> fp16 hi of feat^T, (B, HW2, C)
    ftp = np.ascontiguousarray(feat.transpose(0, 2, 1)).astype(np.float16)
    # word_emb^T hi/lo packed (B, 256, 160) = [hi(77->80) | lo(77->80)]
    wembT = np.ascontiguousarray(word_emb.transpose(0, 2, 1))  # (B, 256, 77)
    whi = wembT.astype(np.float16)
    wlo = (wembT - whi.astype(np.float32)).astype(np.float16)
    wembp = np.zeros((B, WORD_DIM, 2 * NP), dtype=np.float16)
    wembp[:, :, :N_WORDS] = whi
    wembp[:, :, NP : NP + N_WORDS] = wlo
    # W_fc^T hi/lo packed (256, 2048) = [hi(1024) | lo(1024)]
    vT = np.ascontiguousarray(W_fc.T)  # (256, 1024)
    vhi = vT.astype(np.float16)
    vlo = (vT - vhi.astype(np.float32)).astype(np.float16)
    wfcp = np.concatenate([vhi, vlo], axis=1)  # (256, 2048)

    if "nc" not in _CACHE:
        _CACHE["nc"] = _build()
    nc = _CACHE["nc"]

    in_maps = [
        {
            "ftp": ftp[i * BPC : (i + 1) * BPC],
            "wemb": wembp[i * BPC : (i + 1) * BPC],
            "wfc": wfcp,
        }
        for i in range(N_CORES)
    ]
    res = run_bass_kernel_spmd(nc, in_maps, list(range(N_CORES)), **run_kwargs)
    LAST_RESULT = res
    out = np.concatenate([res.results[i]["out"] for i in range(N_CORES)], axis=0)
    # b_fc shifts all logits of a softmax row equally (no effect on A) and
    # adds linearly to the output: out = A @ we0 + b_fc. Exact identity.
    out = out.astype(np.float32) + b_fc.reshape(1, 1, HW2)
    return out.reshape(B, C, H, W).astype(np.float32)


# revision 9
# speedup vs baseline: 1.5653x; 1.0293x over previous
"""Channel-attention kernel for Trainium2 (8 NeuronCores, batch-parallel).

Reference computation per batch b (feat (C, HW2), word_emb (N, D)):
    we0   = word_emb @ W_fc^T                 (N, HW2)
    S     = feat @ we0^T                      (C, N)   [b_fc shifts every logit
                                                        of a row equally -> the
                                                        softmax is invariant]
    A     = softmax(S, axis=-1)
    out   = A @ we0 + b_fc                    (C, HW2) [b_fc added on host]

v4 design (v1 108.6us -> v2 65.5us -> this):
  - feat ships fp16-hi ONLY (half the input DMA); score = ONE fp16 chain
    (wthi^T @ FThi, 8 matmuls) -- the we0-lo correction chain is dropped.
    Emulated numerics on the real seed: scale-rel absmax 1.27e-2 vs the 2e-2
    gate (HW has tracked the emulation within 4%).
  - output stored fp16 (half the output DMA); host casts to fp32.
  - softmax normalization folded into the O-phase: O = E^T @ we0 as f32r
    matmuls with UN-normalized E stationary; per-c sums via a tiny ones
    matmul; 1/sums applied as a per-partition scale during the PSUM->SBUF
    out-copies (ACT + DVE split).
  - we0 transposed once in fp32 (8 PE transposes), hi split from PSUM.
  - wembT/wfcT hi/lo marshalled on host (we0 itself stays 3-chain accurate).
  - head trimmed: wembT0 + wfc-hi dispatched on sync, ft0 + wfc-lo on the
    scalar queue, so batch 0's we0 starts ~3us earlier.

Device dataflow per batch (one NeuronCore handles B/8 = 4 batches):
    we0 psum    = 3 fp16-pair chains wembT^T @ wfcT      (12 mm, ~fp32-exact)
    we0         = ACT copies psum -> SBUF f32r           (O-phase moving)
    we0T psum   = 8 fp32 PE transposes of we0 (bitcast)
    wt hi       = ACT copies we0T psum -> fp16
    S^T         = wthi^T @ FThi                          (8 mm into one PSUM)
    Eh          = exp(0.5*S^T - 48)                      (ACT)
    E           = Eh*Eh -> f32r                          (DVE; = exp(S^T-96))
    per ct(4):  O = E-slice^T @ we0 (f32r, 2 mm), sums = E-slice^T @ ones
                rr = 1/sums (DVE); out fp16 = psum * rr (ACT half, DVE half)
                per-ct out DMA on the scalar queue
"""

import numpy as np

import concourse.bass as bass
import concourse.mybir as mybir
import concourse.tile as tile
from concourse import bacc
from concourse.bass import ds, ts
from concourse.bass_utils import run_bass_kernel_spmd
from concourse.masks import make_identity

B, C, HW2 = 32, 512, 1024
N_WORDS, WORD_DIM = 77, 256
H = W = 32
N_CORES = 8
BPC = B // N_CORES  # batches per core
NP = 80  # N_WORDS padded to a multiple of 16

FP32 = mybir.dt.float32
FP16 = mybir.dt.float16
F32R = mybir.dt.float32r
AF = mybir.ActivationFunctionType

EXP_SCALE = 0.5
EXP_BIAS = -48.0  # exp(0.5*s - 48)^2 == exp(s - 96)

LAST_RESULT = None  # BassKernelResults of the most recent run (for test.py)


def _body(nc, tc, ftp_d, wemb_d, wfc_d, out_d):
    from contextlib import ExitStack

    with ExitStack() as ctx:
        const = ctx.enter_context(tc.tile_pool(name="const", bufs=1))
        big = ctx.enter_context(tc.tile_pool(name="big", bufs=2))
        med = ctx.enter_context(tc.tile_pool(name="med", bufs=2))
        outp = ctx.enter_context(tc.tile_pool(name="outp", bufs=4))
        mm_ps = ctx.enter_context(tc.tile_pool(name="mm_ps", bufs=4, space="PSUM"))
        tp_ps = ctx.enter_context(tc.tile_pool(name="tp_ps", bufs=2, space="PSUM"))
        sc_ps = ctx.enter_context(tc.tile_pool(name="sc_ps", bufs=1, space="PSUM"))
        su_ps = ctx.enter_context(tc.tile_pool(name="su_ps", bufs=1, space="PSUM"))

        ident = const.tile([128, 128], FP32)
        make_identity(nc, ident[:])
        ones_f = const.tile([128, 8], FP32)
        nc.gpsimd.memset(ones_f[:], 1.0)
        ones = const.tile([128, 8], F32R)
        nc.vector.tensor_copy(ones[:], ones_f[:])
        ebias = const.tile([128, 1], FP32)
        nc.gpsimd.memset(ebias[:], EXP_BIAS)

        # W_fc^T hi|lo packed, host-marshalled: (256, 2048) -> (128, 2, 2048)
        vt = const.tile([128, 2, 2048], FP16)

        def load_wemb(b, st):
            # wembT packed (256, 160) = [hi(80) | lo(80)] -> (128, 2, 160)
            wembT = st["wembT"] = med.tile(
                [128, 2, 160], FP16, tag="wembT", name="wembT"
            )
            nc.sync.dma_start(wembT[:], wemb_d[b].rearrange("(c p) x -> p c x", p=128))

        def load_ft(b, st, eng=None):
            # FT hi (hw-partitioned): (1024, 512) -> (128, 8, 512)
            ft = st["ft"] = big.tile([128, 8, 512], FP16, tag="ft", name="ft")
            (eng or nc.sync).dma_start(
                ft[:], ftp_d[b].rearrange("(t p) x -> p t x", p=128)
            )

        def we0_phase(st):
            # we0 = word_emb @ W_fc^T (77, 1024): 3 fp16-pair chains, vhi
            # chains first so batch 0 can start before the vlo DMA lands.
            wembT = st["wembT"]
            whi = (wembT[:, 0, :NP], wembT[:, 1, :NP])
            wlo = (wembT[:, 0, NP:], wembT[:, 1, NP:])
            for half in range(2):
                ps = mm_ps.tile([128, 512], FP32, tag="mm")
                sl = ds(half * 512, 512)
                mms = []
                for dc in range(2):
                    mms.append((whi[dc], dc, 0))
                for dc in range(2):
                    mms.append((whi[dc], dc, 1024))
                    mms.append((wlo[dc], dc, 0))
                for j, (lhsT, dc, base) in enumerate(mms):
                    nc.tensor.matmul(
                        ps[:NP, :],
                        lhsT,
                        vt[:, dc, ds(base + half * 512, 512)],
                        start=(j == 0),
                        stop=(j == 5),
                    )
                if half == 0:
                    we0 = st["we0"] = med.tile(
                        [128, 1024], F32R, tag="we0", name="we0"
                    )
                nc.scalar.copy(st["we0"][:N_WORDS, sl], ps[:N_WORDS, :])

        def wt_phase(st):
            # we0^T (hi only) via 8 fp32 PE transposes; fp16 cast from PSUM
            we0 = st["we0"][:].bitcast(FP32)
            wthi = st["wthi"] = med.tile([128, 8, NP], FP16, tag="wthi", name="wthi")
            for g in range(2):
                tps = tp_ps.tile([128, 4, NP], FP32, tag="tp")
                for j in range(4):
                    nc.tensor.matmul(
                        tps[:, j, :N_WORDS],
                        we0[:N_WORDS, ts(g * 4 + j, 128)],
                        ident[:N_WORDS, :N_WORDS],
                        is_transpose=True,
                        start=(j == 0),
                        stop=(j == 3),
                    )
                nc.scalar.copy(wthi[:, ds(g * 4, 4), :N_WORDS], tps[:, :, :N_WORDS])

        def score(st):
            # S^T = wthi^T @ FThi  (77, 512), 8 matmuls
            ft = st["ft"]
            sps = st["sps"] = sc_ps.tile([128, 512], FP32, tag="sc", name="sps")
            wthi = st["wthi"]
            for kt in range(8):
                nc.tensor.matmul(
                    sps[:NP, :],
                    wthi[:, kt, :],
                    ft[:, kt, :],
                    start=(kt == 0),
                    stop=(kt == 7),
                )

        def soft(st):
            # E = exp(S^T - 96) via exp(0.5*s - 48)^2, kept f32r un-normalized
            sps = st["sps"]
            ehalf = med.tile([128, 512], FP32, tag="ehalf")
            nc.scalar.activation(
                ehalf[:N_WORDS, :],
                sps[:N_WORDS, :],
                AF.Exp,
                bias=ebias[:N_WORDS, :],
                scale=EXP_SCALE,
            )
            eT = st["eT"] = med.tile([128, 512], F32R, tag="eT", name="eT")
            nc.vector.tensor_mul(eT[:N_WORDS, :], ehalf[:N_WORDS, :], ehalf[:N_WORDS, :])

        def o_phase(st, b):
            # O = E-slice^T @ we0 (f32r) + ones-column sums; normalize on copy
            eT, we0 = st["eT"], st["we0"]
            rr = med.tile([128, 4], FP32, tag="rr")
            for ct in range(4):
                e_sl = eT[:N_WORDS, ts(ct, 128)]
                ops0 = mm_ps.tile([128, 512], FP32, tag="mm")
                nc.tensor.matmul(ops0[:], e_sl, we0[:N_WORDS, :512])
                ops1 = mm_ps.tile([128, 512], FP32, tag="mm")
                nc.tensor.matmul(ops1[:], e_sl, we0[:N_WORDS, 512:])
                su = su_ps.tile([128, 8], FP32, tag="su")
                nc.tensor.matmul(su[:], e_sl, ones[:N_WORDS, :])
                rc = rr[:, ct : ct + 1]
                nc.vector.reciprocal(rc, su[:, 0:1])
                ob = outp.tile([128, 1024], FP16, tag="ob")
                nc.scalar.mul(ob[:, :512], ops0[:], rc)
                nc.vector.tensor_scalar_mul(ob[:, 512:], ops1[:], rc)
                nc.scalar.dma_start(out_d[b, ts(ct, 128), :], ob[:])

        # software pipeline: batch b's O phase is emitted behind batch b+1's
        # score so the (in-order) PE queue always has independent work while
        # b's softmax chain runs on ACT/DVE.
        states = {b: {} for b in range(BPC)}
        load_wemb(0, states[0])
        nc.sync.dma_start(
            vt[:, :, :1024], wfc_d[:, :1024].rearrange("(c p) x -> p c x", p=128)
        )
        load_ft(0, states[0], eng=nc.scalar)
        nc.scalar.dma_start(
            vt[:, :, 1024:], wfc_d[:, 1024:].rearrange("(c p) x -> p c x", p=128)
        )
        we0_phase(states[0])
        wt_phase(states[0])
        load_wemb(1, states[1])
        load_ft(1, states[1])
        score(states[0])
        soft(states[0])
        for b in range(1, BPC):
            we0_phase(states[b])
            wt_phase(states[b])
            if b + 1 < BPC:
                load_wemb(b + 1, states[b + 1])
                load_ft(b + 1, states[b + 1])
            score(states[b])
            o_phase(states[b - 1], b - 1)
            soft(states[b])
            del states[b - 1]
        o_phase(states[BPC - 1], BPC - 1)


def _build():
    nc = bacc.Bacc(
        "TRN2",
        target_bir_lowering=False,
        debug=False,
        enable_asserts=False,
        num_devices=N_CORES,
    )
    ftp_d = nc.declare_dram_parameter("ftp", [BPC, HW2, C], FP16, isOutput=False)
    wemb_d = nc.declare_dram_parameter(
        "wemb", [BPC, WORD_DIM, 2 * NP], FP16, isOutput=False
    )
    wfc_d = nc.declare_dram_parameter("wfc", [WORD_DIM, 2048], FP16, isOutput=False)
    out_d = nc.declare_dram_parameter("out", [BPC, C, HW2], FP16, isOutput=True)
    with tile.TileContext(nc) as tc:
        _body(nc, tc, ftp_d, wemb_d, wfc_d, out_d)
    nc.finalize()
    return nc


_CACHE = {}


def kernel(feat, word_emb, W_fc, b_fc, **run_kwargs):
    global LAST_RESULT
    feat = np.asarray(feat, dtype=np.float32).reshape(B, C, HW2)
    word_emb = np.asarray(word_emb, dtype=np.float32)
    W_fc = np.asarray(W_fc, dtype=np.float32)
    b_fc = np.asarray(b_fc, dtype=np.float32)

    # host marshalling (layout/dtype only):
    # feat -> fp16 hi of feat^T, (B, HW2, C)
    ftp = np.ascontiguousarray(feat.transpose(0, 2, 1)).astype(np.float16)
    # word_emb^T hi/lo packed (B, 256, 160) = [hi(77->80) | lo(77->80)]
    wembT = np.ascontiguousarray(word_emb.transpose(0, 2, 1))  # (B, 256, 77)
    whi = wembT.astype(np.float16)
    wlo = (wembT - whi.astype(np.float32)).astype(np.float16)
    wembp = np.zeros((B, WORD_DIM, 2 * NP), dtype=np.float16)
    wembp[:, :, :N_WORDS] = whi
    wembp[:, :, NP : NP + N_WORDS] = wlo
    # W_fc^T hi/lo packed (256, 2048) = [hi(1024) | lo(1024)]
    vT = np.ascontiguousarray(W_fc.T)  # (256, 1024)
    vhi = vT.astype(np.float16)
    vlo = (vT - vhi.astype(np.float32)).astype(np.float16)
    wfcp = np.concatenate([vhi, vlo], axis=1)  # (256, 2048)

    if "nc" not in _CACHE:
        _CACHE["nc"] = _build()
    nc = _CACHE["nc"]

    in_maps = [
        {
            "ftp": ftp[i * BPC : (i + 1) * BPC],
            "wemb": wembp[i * BPC : (i + 1) * BPC],
            "wfc": wfcp,
        }
        for i in range(N_CORES)
    ]
    res = run_bass_kernel_spmd(nc, in_maps, list(range(N_CORES)), **run_kwargs)
    LAST_RESULT = res
    out = np.concatenate([res.results[i]["out"] for i in range(N_CORES)], axis=0)
    # b_fc shifts all logits of a softmax row equally (no effect on A) and
    # adds linearly to the output: out = A @ we0 + b_fc. Exact identity.
    out = out.astype(np.float32) + b_fc.reshape(1, 1, HW2)
    return out.reshape(B, C, H, W).astype(np.float32)
